# revision 39
# baseline (speedup 1.0000x reference)
"""GAT (3-layer) Trainium2 kernel, 8 NeuronCores.

Design:
- Nodes are degree-sorted and snake-assigned to 8 cores; each core owns a
  contiguous block of 6272 "ranks" (6250 real nodes + pads). Edge e belongs to
  the core owning dst[e].
- Per layer: a dense phase computes a per-node table row
  [feat(HD) | el(H) | er(H) | pad] = x @ [W | wel | wer] (el/er folded into the
  matmul on the host: wel = sum_d W[:,h,d]*al[h,d]).
- Edge phase: per 128-node window (K slots/node, uniform K per window across
  cores), one dma_gather brings pair-rows (2 nodes / 1280B row, idx = rank>>1
  fits int16); a parity mask selects the correct half. Slot 0 of each node is a
  self-slot providing er[dst]. Softmax + weighted sum are free-axis DVE
  reduces. Output rows flush to a core-local buffer; an AllGather rebuilds the
  full x for the next layer.
"""

import os
import sys
import types

import numpy as np
import ml_dtypes

import concourse.bass as bass
import concourse.bacc as bacc
import concourse.mybir as mybir
import concourse.tile as tile
from concourse.tile import add_dep_helper
from concourse.bass_utils import run_bass_kernel_spmd

# NTFF profiling shim (axon images lack antenv.axon_hooks)
try:
    from trn_agent_boot.trn_boot import _ntff_profile_via_ctypes
    _hook = _ntff_profile_via_ctypes('/opt/axon/libaxon_pjrt.so')
    _m = types.ModuleType("antenv.axon_hooks")
    _m.get_axon_ntff_profile_hook = lambda: _hook
    sys.modules['antenv.axon_hooks'] = _m
except Exception:
    pass

N_NODES = 50000
N_EDGES = 800000
IN_FEATS = 256
HIDDEN = 32
HEADS = 4
NUM_CLASSES = 40
NEG_SLOPE = 0.2

N_CORES = 8
NPC = 6272            # nodes (ranks) per core, = 49 * 128
NW = NPC // 128       # windows per core = 49
NTOT = N_CORES * NPC  # 50176 padded rank space
F32 = mybir.dt.float32
BF16 = mybir.dt.bfloat16
I16 = mybir.dt.int16
I8 = mybir.dt.uint8

# per-layer: (F_in, HD, H, D, HALF, PAIR)
LAYERS = [
    (256, 128, 4, 32, 192, 384),
    (128, 128, 4, 32, 192, 384),
    (128, 160, 4, 40, 192, 384),
]

LAST_EXEC_NS = [None]
LAST_RES = [None]


def _host_prep(src, dst):
    """Degree-sort nodes, assign to cores, build per-core slot grids."""
    deg = np.bincount(dst, minlength=N_NODES)
    order = np.argsort(-deg, kind="stable")  # desc degree
    # snake-deal to 8 cores for edge balance
    core_nodes = [[] for _ in range(N_CORES)]
    for i, n in enumerate(order):
        r = i // N_CORES
        k = i % N_CORES if (r % 2 == 0) else (N_CORES - 1 - i % N_CORES)
        core_nodes[k].append(n)
    # within each core keep degree-desc order (so windows have uniform degree)
    rank_of = np.full(N_NODES, -1, np.int64)
    core_lists = []
    for k in range(N_CORES):
        nodes = np.array(core_nodes[k], np.int64)
        d = deg[nodes]
        o = np.argsort(-d, kind="stable")
        nodes = nodes[o]
        core_lists.append(nodes)
        rank_of[nodes] = k * NPC + np.arange(len(nodes))
    # CSR of incoming edges by destination rank
    dst_rank = rank_of[dst]
    src_rank = rank_of[src]
    eorder = np.argsort(dst_rank, kind="stable")
    dst_rank_s = dst_rank[eorder]
    src_rank_s = src_rank[eorder]
    # row pointer over the padded rank space
    cnt = np.bincount(dst_rank_s, minlength=NTOT)
    rp = np.zeros(NTOT + 1, np.int64)
    np.cumsum(cnt, out=rp[1:])
    # per (core, window) K = max degree in window (+1 self slot)
    degr = np.zeros(NTOT, np.int64)
    degr[:len(rank_of)] = 0
    degr = cnt  # in-degree per rank
    Ks = np.zeros(NW, np.int64)
    for w in range(NW):
        m = 0
        for k in range(N_CORES):
            lo = k * NPC + w * 128
            m = max(m, int(degr[lo:lo + 128].max()))
        Ks[w] = m + 1  # +1 self slot
    # build per-core idx/mask streams
    S_tot = int(Ks.sum())  # slots per node-column total
    idx16 = np.zeros((N_CORES, 128, 8 * S_tot), np.int16)
    msk_par = np.zeros((N_CORES, 128, S_tot), np.int16)
    msk_ex = np.zeros((N_CORES, 128, S_tot), np.float32)
    streams = np.zeros((N_CORES, 128 * S_tot), np.int64)  # slot-pos -> rank
    offs = np.zeros(NW + 1, np.int64)
    np.cumsum(Ks, out=offs[1:])
    for k in range(N_CORES):
        for w in range(NW):
            Kp = int(Ks[w])
            base = k * NPC + w * 128
            # slot grid [128 nodes, Kp]
            grid = np.zeros((128, Kp), np.int64)  # src ranks; self slot at 0
            mex = np.zeros((128, Kp), np.float32)
            for p in range(128):
                r = base + p
                grid[p, 0] = r  # self slot (gives er[dst])
                d = int(degr[r])
                e0 = rp[r]
                grid[p, 1:1 + d] = src_rank_s[e0:e0 + d]
                mex[p, 1:1 + d] = 1.0
                # pads keep idx 0 (valid row), mex 0
            mex[:, 0] = 1e-30  # self slot: keeps denom > 0, removes max guard
            par = (grid & 1).astype(np.int16)
            pair = (grid >> 1).astype(np.int16)
            msk_par[k, :, offs[w]:offs[w + 1]] = par
            msk_ex[k, :, offs[w]:offs[w + 1]] = mex
            # gather list position i = s*128 + n  -> wrapped int16 layout
            flat = pair.T.reshape(-1)  # [Kp*128] position i = s*128+n
            streams[k, 128 * offs[w]:128 * offs[w + 1]] = grid.T.reshape(-1)
            wrapped = flat.reshape(8 * Kp, 16).T  # [16, 8Kp]
            blk = np.tile(wrapped, (8, 1))  # [128, 8Kp]
            idx16[k, :, 8 * offs[w]:8 * offs[w + 1]] = blk
    return core_lists, rank_of, Ks, offs, S_tot, idx16, msk_par, msk_ex, streams


def _fold_w(W, al, ar, HD, H, D, F_in):
    """[W | wel | wer] bf16, where el = x @ wel etc."""
    Wf = np.asarray(W, np.float32)
    alf = np.asarray(al, np.float32).reshape(H, D)
    arf = np.asarray(ar, np.float32).reshape(H, D)
    W3 = Wf.reshape(F_in, H, D)
    wel = np.einsum('fhd,hd->fh', W3, alf)
    wer = np.einsum('fhd,hd->fh', W3, arf)
    return np.concatenate([Wf, wel, wer], axis=1).astype(ml_dtypes.bfloat16)


def _build_program(Ks, offs, S_tot, trace_collect):
    nc = bacc.Bacc("TRN2", target_bir_lowering=False,
                   detect_race_conditions=False,
                   num_swdge_queues=4)
    AG = mybir.AluOpType

    x0sT = nc.dram_tensor("x0sT", [256, 128 * S_tot], BF16,
                          kind="ExternalInput")
    Wt = [nc.dram_tensor(f"w{L}", [LAYERS[L][0], LAYERS[L][1] + 8], BF16,
                         kind="ExternalInput") for L in range(3)]
    idx_in = nc.dram_tensor("idx16", [128, 8 * S_tot], I16, kind="ExternalInput")
    mpar_in = nc.dram_tensor("mpar", [128, S_tot], I16, kind="ExternalInput")
    mex_in = nc.dram_tensor("mex", [128, S_tot], BF16, kind="ExternalInput")
    ident_in = nc.dram_tensor("ident", [128, 128], BF16, kind="ExternalInput")

    nl = 3
    tabs = [None] + [nc.dram_tensor(f"tab{L}", [NTOT, LAYERS[L][4]], BF16)
                     for L in (1, 2)]
    xfull = [None] + [nc.dram_tensor(f"x{i}", [NTOT, 128], BF16,
                                     addr_space="Shared") if i < nl
                      else None for i in (1, 2)]
    xloc = [None] + [nc.dram_tensor(f"x{i}loc", [NPC, 128], BF16)
                     for i in (1, 2)]
    out_t = nc.dram_tensor("logits", [NPC, NUM_CLASSES], F32,
                           kind="ExternalOutput")

    with tile.TileContext(nc) as tc:
        with (tc.tile_pool(name="sb", bufs=3) as sb,
              tc.tile_pool(name="sbG", bufs=3) as sbG,
              tc.tile_pool(name="sm", bufs=3) as sm,
              tc.tile_pool(name="sbd", bufs=2) as sbd,
              tc.tile_pool(name="sbw", bufs=1) as sbw,
              tc.tile_pool(name="ps", bufs=4, space="PSUM") as ps):
            alph = sbw.tile([128, 1], F32, tag="alph")
            nc.vector.memset(alph[:], NEG_SLOPE)
            # preload the full per-layer (graph-constant) index/mask streams
            it_all = sbw.tile([128, 8 * S_tot], I16, tag="it_all")
            nc.sync.dma_start(it_all[:], idx_in[:, :])
            mp_all = sbw.tile([128, S_tot], I16, tag="mp_all")
            nc.sync.dma_start(mp_all[:], mpar_in[:, :])
            me_all = sbw.tile([128, S_tot], BF16, tag="me_all")
            nc.sync.dma_start(me_all[:], mex_in[:, :])
            ident = sbw.tile([128, 128], BF16, tag="ident")
            nc.sync.dma_start(ident[:], ident_in[:, :])
            prev_nop = None
            n_layers = 3
            for L in range(n_layers):
                F_in, HD, H, D, HALF, PAIR = LAYERS[L]
                nch = F_in // 128
                WD = HD + 8
                # ---- dense: tab[L] rows = x @ [W|wel|wer] (layers 1,2) ----
                wsb = sbw.tile([128, nch * WD], BF16, tag=f"w{L}")
                for c in range(nch):
                    nc.sync.dma_start(wsb[:, c * WD:(c + 1) * WD],
                                      Wt[L][c * 128:(c + 1) * 128, :])
                if L > 0:
                    xsrc = xfull[L]
                    dense_writes = []
                    GRP = 896
                    for g in range(NTOT // GRP):
                        xts = []
                        for c in range(nch):
                            xt = sbd.tile([128, GRP], BF16, tag=f"xt{c}")
                            ld = nc.sync.dma_start_transpose(
                                xt[:], xsrc[g * GRP:(g + 1) * GRP,
                                            c * 128:(c + 1) * 128])
                            if prev_nop is not None:
                                add_dep_helper(ld.ins, prev_nop.ins)
                            xts.append(xt)
                        rows = sbd.tile([128, 7 * HALF], BF16, tag="rows")
                        for t in range(7):
                            acc = ps.tile([128, WD], F32, tag="dacc")
                            for c in range(nch):
                                nc.tensor.matmul(
                                    acc[:],
                                    lhsT=xts[c][:, t * 128:(t + 1) * 128],
                                    rhs=wsb[:, c * WD:(c + 1) * WD],
                                    start=(c == 0), stop=(c == nch - 1))
                            if t % 2 == 0:
                                nc.scalar.copy(
                                    rows[:, t * HALF:t * HALF + WD], acc[:])
                            else:
                                nc.vector.tensor_copy(
                                    rows[:, t * HALF:t * HALF + WD], acc[:])
                        wr = nc.sync.dma_start(
                            tabs[L][g * GRP:(g + 1) * GRP, :].rearrange(
                                "(t p) h -> p t h", p=128),
                            rows[:].rearrange("p (t h) -> p t h", t=7))
                        dense_writes.append(wr)
                    nop_d = nc.gpsimd.engine_nop()
                    for wri in dense_writes:
                        add_dep_helper(nop_d.ins, wri.ins)
                    tabap = tabs[L][:, :].rearrange(
                        "(a two) h -> a (two h)", two=2)
                # ---- edge phase ----
                flush_writes = []
                for w in range(NW):
                    Kp = int(Ks[w])
                    NIDX = 128 * Kp
                    sel = sb.tile([128, Kp * WD], BF16, tag="sel")
                    sv = sel[:].rearrange("p (s e) -> p s e", e=WD)
                    if L == 0:
                        # host-pregathered raw-x stream -> per-slot PE matmul
                        s0 = sbG.tile([128, 128 * Kp], BF16, tag="G")
                        nc.sync.dma_start(
                            s0[:], x0sT[0:128,
                                        128 * int(offs[w]):128 * int(offs[w + 1])])
                        s1 = sbG.tile([128, 128 * Kp], BF16, tag="G")
                        nc.sync.dma_start(
                            s1[:], x0sT[128:256,
                                        128 * int(offs[w]):128 * int(offs[w + 1])])
                        for s in range(Kp):
                            acc = ps.tile([128, WD], F32, tag="dacc")
                            nc.tensor.matmul(
                                acc[:], lhsT=s0[:, s * 128:(s + 1) * 128],
                                rhs=wsb[:, 0:WD], start=True, stop=False)
                            nc.tensor.matmul(
                                acc[:], lhsT=s1[:, s * 128:(s + 1) * 128],
                                rhs=wsb[:, WD:2 * WD], start=False, stop=True)
                            if s % 2 == 0:
                                nc.scalar.copy(
                                    sel[:, s * WD:(s + 1) * WD], acc[:])
                            else:
                                nc.vector.tensor_copy(
                                    sel[:, s * WD:(s + 1) * WD], acc[:])
                    else:
                        G = sbG.tile([128, Kp * PAIR], BF16, tag="G")
                        Gv = G[:].rearrange("p (s e) -> p s e", e=PAIR)
                        # split into 4 quarter-gathers, one per SWDGE queue, so
                        # the ring holds several in flight (gen overlaps drain)
                        qt = (Kp + 3) // 4
                        bounds = [min(i * qt, Kp) for i in range(5)]
                        parts = tuple((bounds[i], bounds[i + 1])
                                      for i in range(4))
                        for j, (sa, sz) in enumerate(parts):
                            if sz <= sa:
                                continue
                            n = sz - sa
                            nc.gpsimd.dma_gather(
                                out_ap=Gv[:, sa:sz, :],
                                in_ap=tabap,
                                idxs_ap=it_all[:, 8 * (int(offs[w]) + sa):
                                               8 * (int(offs[w]) + sz)],
                                num_idxs=128 * n, num_idxs_reg=128 * n,
                                elem_size=PAIR, single_packet=False,
                                queue_num=(4 * w + j) % 4)
                            svh = sv[:, sa:sz, :]
                            nc.scalar.copy(svh, Gv[:, sa:sz, 0:WD])
                            mp = mp_all[:, int(offs[w]) + sa:
                                        int(offs[w]) + sz]
                            nc.vector.copy_predicated(
                                svh,
                                mp[:, :, None].to_broadcast([128, n, WD]),
                                Gv[:, sa:sz, HALF:HALF + WD])
                    # softmax path
                    ts_ = sm.tile([128, Kp * H], F32, tag="ts")
                    tv = ts_[:].rearrange("p (s h) -> p s h", h=H)
                    nc.vector.tensor_tensor(
                        out=tv, in0=sv[:, :, HD:HD + H],
                        in1=sv[:, 0:1, HD + H:HD + 2 * H].to_broadcast(
                            [128, Kp, H]),
                        op=AG.add)
                    tm = sm.tile([128, Kp * H], F32, tag="tm")
                    nc.scalar.activation(tm[:], ts_[:],
                                         mybir.ActivationFunctionType.Prelu,
                                         alpha=alph[:])
                    ex = sm.tile([128, Kp * H], F32, tag="ex")
                    nc.scalar.activation(ex[:], tm[:],
                                         mybir.ActivationFunctionType.Exp)
                    me = me_all[:, int(offs[w]):int(offs[w + 1])]
                    exv = ex[:].rearrange("p (s h) -> p s h", h=H)
                    nc.vector.tensor_tensor(
                        out=exv, in0=exv,
                        in1=me[:, :, None].to_broadcast([128, Kp, H]),
                        op=AG.mult)
                    den = sm.tile([128, H], F32, tag="den")
                    nc.vector.tensor_reduce(
                        den[:], ex[:].rearrange("p (s h) -> p h s", h=H),
                        axis=mybir.AxisListType.X, op=AG.add)
                    rec = sm.tile([128, H], F32, tag="rec")
                    nc.vector.reciprocal(rec[:], den[:])
                    # messages
                    msg = sb.tile([128, Kp * HD], BF16, tag="msg")
                    nc.vector.tensor_tensor(
                        out=msg[:].rearrange("p (s h d) -> p s h d", h=H, d=D),
                        in0=sv[:, :, 0:HD].rearrange(
                            "p s (h d) -> p s h d", h=H),
                        in1=exv[:, :, :, None].to_broadcast([128, Kp, H, D]),
                        op=AG.mult)
                    num = ps.tile([128, HD], F32, tag="nacc")
                    for k in range(Kp):
                        nc.tensor.matmul(
                            num[:], lhsT=ident[:],
                            rhs=msg[:, k * HD:(k + 1) * HD],
                            start=(k == 0), stop=(k == Kp - 1))
                    outw = sm.tile([128, HD], F32, tag="outw")
                    nc.vector.tensor_tensor(
                        out=outw[:].rearrange("p (h d) -> p h d", h=H),
                        in0=num[:].rearrange("p (h d) -> p h d", h=H),
                        in1=rec[:, :, None].to_broadcast([128, H, D]),
                        op=AG.mult)
                    if L < 2:
                        r1 = sm.tile([128, HD], F32, tag="r1")
                        nc.scalar.activation(r1[:], outw[:],
                                             mybir.ActivationFunctionType.Relu)
                        m1 = sm.tile([128, HD], F32, tag="m1")
                        nc.scalar.activation(m1[:], outw[:],
                                             mybir.ActivationFunctionType.Relu,
                                             scale=-1.0)
                        em = sm.tile([128, HD], F32, tag="em")
                        nc.scalar.activation(em[:], m1[:],
                                             mybir.ActivationFunctionType.Exp,
                                             scale=-1.0)
                        xl = sm.tile([128, HD], BF16, tag="xl")
                        nc.vector.scalar_tensor_tensor(
                            out=xl[:], in0=em[:], scalar=-1.0, in1=r1[:],
                            op0=AG.add, op1=AG.add)
                        fw = nc.sync.dma_start(
                            xloc[L + 1][w * 128:(w + 1) * 128, :], xl[:])
                    else:
                        prod = sm.tile([128, HD], F32, tag="prod")
                        nc.vector.tensor_tensor(
                            out=prod[:].rearrange("p (h c) -> p h c", h=H),
                            in0=num[:].rearrange("p (h c) -> p h c", h=H),
                            in1=rec[:, :, None].to_broadcast(
                                [128, H, NUM_CLASSES]),
                            op=AG.mult)
                        lg = sm.tile([128, NUM_CLASSES], F32, tag="lg")
                        nc.vector.tensor_reduce(
                            lg[:], prod[:].rearrange(
                                "p (h c) -> p c h", h=H),
                            axis=mybir.AxisListType.X, op=AG.add)
                        nc.vector.tensor_scalar_mul(lg[:], lg[:], 1.0 / H)
                        fw = nc.sync.dma_start(
                            out_t[w * 128:(w + 1) * 128, :], lg[:])
                    flush_writes.append(fw)
                nop_e = nc.gpsimd.engine_nop()
                for fwi in flush_writes:
                    add_dep_helper(nop_e.ins, fwi.ins)
                if L < n_layers - 1:
                    cc = nc.gpsimd.collective_compute(
                        "AllGather", AG.bypass,
                        [[i for i in range(N_CORES)]],
                        ins=[xloc[L + 1][:, :]], outs=[xfull[L + 1][:, :]])
                    nop_c = nc.vector.engine_nop()
                    add_dep_helper(nop_c.ins, cc.ins)
                    prev_nop = nop_c
                else:
                    prev_nop = nop_e
    nc.finalize()
    return nc


def kernel(**inputs):
    h = np.asarray(inputs["h"], np.float32)
    src = np.asarray(inputs["src"]).astype(np.int64)
    dst = np.asarray(inputs["dst"]).astype(np.int64)

    (core_lists, rank_of, Ks, offs, S_tot, idx16, mpar, mex,
     streams) = _host_prep(src, dst)

    # x0: h rows in rank order, bf16, padded
    x0 = np.zeros((NTOT, 256), ml_dtypes.bfloat16)
    for k in range(N_CORES):
        nodes = core_lists[k]
        x0[k * NPC:k * NPC + len(nodes)] = h[nodes].astype(ml_dtypes.bfloat16)

    # host pre-gather of the layer-0 slot stream, transposed for PE lhsT
    x0sT_hosts = [np.ascontiguousarray(x0[streams[k]].T)
                  for k in range(N_CORES)]
    Ws = [
        _fold_w(inputs["W0"], inputs["al0"], inputs["ar0"], 128, 4, 32, 256),
        _fold_w(inputs["W1"], inputs["al1"], inputs["ar1"], 128, 4, 32, 128),
        _fold_w(inputs["W2"], inputs["al2"], inputs["ar2"], 160, 4, 40, 128),
    ]

    nc = _build_program(Ks, offs, S_tot, None)

    in_maps = []
    for k in range(N_CORES):
        m = {"x0sT": x0sT_hosts[k], "w0": Ws[0], "w1": Ws[1],
             "w2": Ws[2], "idx16": idx16[k], "mpar": mpar[k],
             "mex": mex[k].astype(ml_dtypes.bfloat16),
             "ident": np.eye(128, dtype=ml_dtypes.bfloat16)}
        in_maps.append(m)

    trace = bool(int(os.environ.get("GAT_TRACE", "0")))
    res = run_bass_kernel_spmd(nc, in_maps, core_ids=list(range(N_CORES)),
                               trace=trace)
    LAST_EXEC_NS[0] = res.exec_time_ns
    LAST_RES[0] = res

    out = np.zeros((N_NODES, NUM_CLASSES), np.float32)
    for k in range(N_CORES):
        nodes = core_lists[k]
        out[nodes] = res.results[k]["logits"][:len(nodes)]
    return out



# revision 45
# speedup vs baseline: 1.2609x; 1.2609x over previous
"""GAT (3-layer) Trainium2 kernel, 8 NeuronCores.

Design:
- Nodes are degree-sorted and snake-assigned to 8 cores; each core owns a
  contiguous block of 6272 "ranks" (6250 real nodes + pads). Edge e belongs to
  the core owning dst[e].
- Per layer: a dense phase computes a per-node table row
  [feat(HD) | el(H) | er(H) | pad] = x @ [W | wel | wer] (el/er folded into the
  matmul on the host: wel = sum_d W[:,h,d]*al[h,d]).
- Edge phase: per 128-node window (K slots/node, uniform K per window across
  cores), one dma_gather brings pair-rows (2 nodes / 1280B row, idx = rank>>1
  fits int16); a parity mask selects the correct half. Slot 0 of each node is a
  self-slot providing er[dst]. Softmax + weighted sum are free-axis DVE
  reduces. Output rows flush to a core-local buffer; an AllGather rebuilds the
  full x for the next layer.
"""

import os
import sys
import types

import numpy as np
import ml_dtypes

import concourse.bass as bass
import concourse.bacc as bacc
import concourse.mybir as mybir
import concourse.tile as tile
from concourse.tile import add_dep_helper
from concourse.bass_utils import run_bass_kernel_spmd

# NTFF profiling shim (axon images lack antenv.axon_hooks)
try:
    from trn_agent_boot.trn_boot import _ntff_profile_via_ctypes
    _hook = _ntff_profile_via_ctypes('/opt/axon/libaxon_pjrt.so')
    _m = types.ModuleType("antenv.axon_hooks")
    _m.get_axon_ntff_profile_hook = lambda: _hook
    sys.modules['antenv.axon_hooks'] = _m
except Exception:
    pass

N_NODES = 50000
N_EDGES = 800000
IN_FEATS = 256
HIDDEN = 32
HEADS = 4
NUM_CLASSES = 40
NEG_SLOPE = 0.2

N_CORES = 8
NPC = 6272            # nodes (ranks) per core, = 49 * 128
NW = NPC // 128       # windows per core = 49
NTOT = N_CORES * NPC  # 50176 padded rank space
F32 = mybir.dt.float32
BF16 = mybir.dt.bfloat16
I16 = mybir.dt.int16
I8 = mybir.dt.uint8

# per-layer: (F_in, HD, H, D, HALF, PAIR)
LAYERS = [
    (256, 128, 4, 32, 192, 384),
    (128, 128, 4, 32, 192, 384),
    (128, 160, 4, 40, 192, 384),
]

LAST_EXEC_NS = [None]
LAST_RES = [None]


def _host_prep(src, dst):
    """Degree-sort nodes, assign to cores, build per-core slot grids."""
    deg = np.bincount(dst, minlength=N_NODES)
    order = np.argsort(-deg, kind="stable")  # desc degree
    # snake-deal to 8 cores for edge balance
    core_nodes = [[] for _ in range(N_CORES)]
    for i, n in enumerate(order):
        r = i // N_CORES
        k = i % N_CORES if (r % 2 == 0) else (N_CORES - 1 - i % N_CORES)
        core_nodes[k].append(n)
    # within each core keep degree-desc order (so windows have uniform degree)
    rank_of = np.full(N_NODES, -1, np.int64)
    core_lists = []
    for k in range(N_CORES):
        nodes = np.array(core_nodes[k], np.int64)
        d = deg[nodes]
        o = np.argsort(-d, kind="stable")
        nodes = nodes[o]
        core_lists.append(nodes)
        rank_of[nodes] = k * NPC + np.arange(len(nodes))
    # CSR of incoming edges by destination rank
    dst_rank = rank_of[dst]
    src_rank = rank_of[src]
    eorder = np.argsort(dst_rank, kind="stable")
    dst_rank_s = dst_rank[eorder]
    src_rank_s = src_rank[eorder]
    # row pointer over the padded rank space
    cnt = np.bincount(dst_rank_s, minlength=NTOT)
    rp = np.zeros(NTOT + 1, np.int64)
    np.cumsum(cnt, out=rp[1:])
    # per (core, window) K = max degree in window (+1 self slot)
    degr = np.zeros(NTOT, np.int64)
    degr[:len(rank_of)] = 0
    degr = cnt  # in-degree per rank
    Ks = np.zeros(NW, np.int64)
    for w in range(NW):
        m = 0
        for k in range(N_CORES):
            lo = k * NPC + w * 128
            m = max(m, int(degr[lo:lo + 128].max()))
        Ks[w] = m + 1  # +1 self slot
    # build per-core idx/mask streams
    S_tot = int(Ks.sum())  # slots per node-column total
    idx16 = np.zeros((N_CORES, 128, 8 * S_tot), np.int16)
    msk_par = np.zeros((N_CORES, 128, S_tot), np.int16)
    msk_ex = np.zeros((N_CORES, 128, S_tot), np.float32)
    streams = np.zeros((N_CORES, 128 * S_tot), np.int64)  # slot-pos -> rank
    offs = np.zeros(NW + 1, np.int64)
    np.cumsum(Ks, out=offs[1:])
    for k in range(N_CORES):
        for w in range(NW):
            Kp = int(Ks[w])
            base = k * NPC + w * 128
            # slot grid [128 nodes, Kp]
            grid = np.zeros((128, Kp), np.int64)  # src ranks; self slot at 0
            mex = np.zeros((128, Kp), np.float32)
            for p in range(128):
                r = base + p
                grid[p, 0] = r  # self slot (gives er[dst])
                d = int(degr[r])
                e0 = rp[r]
                grid[p, 1:1 + d] = src_rank_s[e0:e0 + d]
                mex[p, 1:1 + d] = 1.0
                # pads keep idx 0 (valid row), mex 0
            mex[:, 0] = 1e-30  # self slot: keeps denom > 0, removes max guard
            par = (grid & 1).astype(np.int16)
            pair = (grid >> 1).astype(np.int16)
            msk_par[k, :, offs[w]:offs[w + 1]] = par
            msk_ex[k, :, offs[w]:offs[w + 1]] = mex
            # gather list position i = s*128 + n  -> wrapped int16 layout
            flat = pair.T.reshape(-1)  # [Kp*128] position i = s*128+n
            streams[k, 128 * offs[w]:128 * offs[w + 1]] = grid.T.reshape(-1)
            wrapped = flat.reshape(8 * Kp, 16).T  # [16, 8Kp]
            blk = np.tile(wrapped, (8, 1))  # [128, 8Kp]
            idx16[k, :, 8 * offs[w]:8 * offs[w + 1]] = blk
    return core_lists, rank_of, Ks, offs, S_tot, idx16, msk_par, msk_ex, streams


def _fold_w(W, al, ar, HD, H, D, F_in):
    """[W | wel | wer] bf16, where el = x @ wel etc."""
    Wf = np.asarray(W, np.float32)
    alf = np.asarray(al, np.float32).reshape(H, D)
    arf = np.asarray(ar, np.float32).reshape(H, D)
    W3 = Wf.reshape(F_in, H, D)
    wel = np.einsum('fhd,hd->fh', W3, alf)
    wer = np.einsum('fhd,hd->fh', W3, arf)
    return np.concatenate([Wf, wel, wer], axis=1).astype(ml_dtypes.bfloat16)


def _build_program(Ks, offs, S_tot, trace_collect):
    nc = bacc.Bacc("TRN2", target_bir_lowering=False,
                   detect_race_conditions=False,
                   num_swdge_queues=4)
    AG = mybir.AluOpType

    x0sT = nc.dram_tensor("x0sT", [256, 128 * S_tot], BF16,
                          kind="ExternalInput")
    Wt = [nc.dram_tensor(f"w{L}", [LAYERS[L][0], LAYERS[L][1] + 8], BF16,
                         kind="ExternalInput") for L in range(3)]
    idx_in = nc.dram_tensor("idx16", [128, 8 * S_tot], I16, kind="ExternalInput")
    mpar_in = nc.dram_tensor("mpar", [128, S_tot], I16, kind="ExternalInput")
    mex_in = nc.dram_tensor("mex", [128, S_tot], BF16, kind="ExternalInput")
    ident_in = nc.dram_tensor("ident", [128, 128], BF16, kind="ExternalInput")

    nl = 3
    tabs = [None] + [nc.dram_tensor(f"tab{L}", [NTOT, LAYERS[L][4]], BF16)
                     for L in (1, 2)]
    # transposed inter-layer activations: xfT rows = core*128 + feat,
    # cols = node offset within that core's block
    xfull = [None] + [nc.dram_tensor(f"x{i}T", [N_CORES * 128, NPC], BF16,
                                     addr_space="Shared") if i < nl
                      else None for i in (1, 2)]
    xloc = [None] + [nc.dram_tensor(f"x{i}locT", [128, NPC], BF16)
                     for i in (1, 2)]
    out_t = nc.dram_tensor("logits", [NPC, NUM_CLASSES], F32,
                           kind="ExternalOutput")

    with tile.TileContext(nc) as tc:
        with (tc.tile_pool(name="sb", bufs=2) as sb,
              tc.tile_pool(name="sbG", bufs=3) as sbG,
              tc.tile_pool(name="sm", bufs=2) as sm,
              tc.tile_pool(name="sbd", bufs=2) as sbd,
              tc.tile_pool(name="sbw", bufs=1) as sbw,
              tc.tile_pool(name="ps", bufs=3, space="PSUM") as ps,
              tc.tile_pool(name="psT", bufs=2, space="PSUM") as psT):
            alph = sbw.tile([128, 1], F32, tag="alph")
            nc.vector.memset(alph[:], NEG_SLOPE)
            # preload the full per-layer (graph-constant) index/mask streams
            it_all = sbw.tile([128, 8 * S_tot], I16, tag="it_all")
            nc.sync.dma_start(it_all[:], idx_in[:, :])
            mp_all = sbw.tile([128, S_tot], I16, tag="mp_all")
            nc.sync.dma_start(mp_all[:], mpar_in[:, :])
            me_all = sbw.tile([128, S_tot], BF16, tag="me_all")
            nc.sync.dma_start(me_all[:], mex_in[:, :])
            ident = sbw.tile([128, 128], BF16, tag="ident")
            nc.sync.dma_start(ident[:], ident_in[:, :])
            prev_nop = None
            n_layers = 3
            for L in range(n_layers):
                F_in, HD, H, D, HALF, PAIR = LAYERS[L]
                nch = F_in // 128
                WD = HD + 8
                # ---- dense: tab[L] rows = x @ [W|wel|wer] (layers 1,2) ----
                wsb = sbw.tile([128, nch * WD], BF16, tag=f"w{L}")
                for c in range(nch):
                    nc.sync.dma_start(wsb[:, c * WD:(c + 1) * WD],
                                      Wt[L][c * 128:(c + 1) * 128, :])
                if L > 0:
                    xsrc = xfull[L]
                    dense_writes = []
                    GRP = 896
                    for g in range(NTOT // GRP):
                        blk, off = g // 7, (g % 7) * GRP
                        xts = []
                        for c in range(nch):
                            xt = sbd.tile([128, GRP], BF16, tag=f"xt{c}")
                            ld = nc.sync.dma_start(
                                xt[:], xsrc[blk * 128:(blk + 1) * 128,
                                            off:off + GRP])
                            if prev_nop is not None:
                                add_dep_helper(ld.ins, prev_nop.ins)
                            xts.append(xt)
                        rows = sbd.tile([128, 7 * HALF], BF16, tag="rows")
                        for t in range(7):
                            acc = ps.tile([128, WD], F32, tag="dacc")
                            for c in range(nch):
                                nc.tensor.matmul(
                                    acc[:],
                                    lhsT=xts[c][:, t * 128:(t + 1) * 128],
                                    rhs=wsb[:, c * WD:(c + 1) * WD],
                                    start=(c == 0), stop=(c == nch - 1))
                            if t % 2 == 0:
                                nc.scalar.copy(
                                    rows[:, t * HALF:t * HALF + WD], acc[:])
                            else:
                                nc.vector.tensor_copy(
                                    rows[:, t * HALF:t * HALF + WD], acc[:])
                        wr = nc.sync.dma_start(
                            tabs[L][g * GRP:(g + 1) * GRP, :].rearrange(
                                "(t p) h -> p t h", p=128),
                            rows[:].rearrange("p (t h) -> p t h", t=7))
                        dense_writes.append(wr)
                    nop_d = nc.gpsimd.engine_nop()
                    for wri in dense_writes:
                        add_dep_helper(nop_d.ins, wri.ins)
                    tabap = tabs[L][:, :].rearrange(
                        "(a two) h -> a (two h)", two=2)
                # ---- edge phase ----
                flush_writes = []
                for w in range(NW):
                    Kp = int(Ks[w])
                    NIDX = 128 * Kp
                    sel = sb.tile([128, Kp * WD], BF16, tag="sel")
                    sv = sel[:].rearrange("p (s e) -> p s e", e=WD)
                    if L == 0:
                        # host-pregathered raw-x stream -> per-slot PE matmul
                        s0 = sbG.tile([128, 128 * Kp], BF16, tag="G")
                        nc.sync.dma_start(
                            s0[:], x0sT[0:128,
                                        128 * int(offs[w]):128 * int(offs[w + 1])])
                        s1 = sbG.tile([128, 128 * Kp], BF16, tag="G")
                        nc.sync.dma_start(
                            s1[:], x0sT[128:256,
                                        128 * int(offs[w]):128 * int(offs[w + 1])])
                        for s in range(Kp):
                            acc = ps.tile([128, WD], F32, tag="dacc")
                            nc.tensor.matmul(
                                acc[:], lhsT=s0[:, s * 128:(s + 1) * 128],
                                rhs=wsb[:, 0:WD], start=True, stop=False)
                            nc.tensor.matmul(
                                acc[:], lhsT=s1[:, s * 128:(s + 1) * 128],
                                rhs=wsb[:, WD:2 * WD], start=False, stop=True)
                            if s % 2 == 0:
                                nc.scalar.copy(
                                    sel[:, s * WD:(s + 1) * WD], acc[:])
                            else:
                                nc.vector.tensor_copy(
                                    sel[:, s * WD:(s + 1) * WD], acc[:])
                    else:
                        G = sbG.tile([128, Kp * PAIR], BF16, tag="G")
                        Gv = G[:].rearrange("p (s e) -> p s e", e=PAIR)
                        # split into 4 quarter-gathers, one per SWDGE queue, so
                        # the ring holds several in flight (gen overlaps drain)
                        qt = (Kp + 3) // 4
                        bounds = [min(i * qt, Kp) for i in range(5)]
                        parts = tuple((bounds[i], bounds[i + 1])
                                      for i in range(4))
                        for j, (sa, sz) in enumerate(parts):
                            if sz <= sa:
                                continue
                            n = sz - sa
                            nc.gpsimd.dma_gather(
                                out_ap=Gv[:, sa:sz, :],
                                in_ap=tabap,
                                idxs_ap=it_all[:, 8 * (int(offs[w]) + sa):
                                               8 * (int(offs[w]) + sz)],
                                num_idxs=128 * n, num_idxs_reg=128 * n,
                                elem_size=PAIR, single_packet=False,
                                queue_num=(4 * w + j) % 4)
                            svh = sv[:, sa:sz, :]
                            nc.scalar.copy(svh, Gv[:, sa:sz, 0:WD])
                            mp = mp_all[:, int(offs[w]) + sa:
                                        int(offs[w]) + sz]
                            nc.vector.copy_predicated(
                                svh,
                                mp[:, :, None].to_broadcast([128, n, WD]),
                                Gv[:, sa:sz, HALF:HALF + WD])
                    # softmax path
                    ts_ = sm.tile([128, Kp * H], F32, tag="ts")
                    tv = ts_[:].rearrange("p (s h) -> p s h", h=H)
                    nc.vector.tensor_tensor(
                        out=tv, in0=sv[:, :, HD:HD + H],
                        in1=sv[:, 0:1, HD + H:HD + 2 * H].to_broadcast(
                            [128, Kp, H]),
                        op=AG.add)
                    tm = sm.tile([128, Kp * H], F32, tag="tm")
                    nc.scalar.activation(tm[:], ts_[:],
                                         mybir.ActivationFunctionType.Prelu,
                                         alpha=alph[:])
                    ex = sm.tile([128, Kp * H], F32, tag="ex")
                    nc.scalar.activation(ex[:], tm[:],
                                         mybir.ActivationFunctionType.Exp)
                    me = me_all[:, int(offs[w]):int(offs[w + 1])]
                    exv = ex[:].rearrange("p (s h) -> p s h", h=H)
                    nc.vector.tensor_tensor(
                        out=exv, in0=exv,
                        in1=me[:, :, None].to_broadcast([128, Kp, H]),
                        op=AG.mult)
                    den = sm.tile([128, H], F32, tag="den")
                    nc.vector.tensor_reduce(
                        den[:], ex[:].rearrange("p (s h) -> p h s", h=H),
                        axis=mybir.AxisListType.X, op=AG.add)
                    rec = sm.tile([128, H], F32, tag="rec")
                    nc.vector.reciprocal(rec[:], den[:])
                    # messages
                    msg = sb.tile([128, Kp * HD], BF16, tag="msg")
                    nc.vector.tensor_tensor(
                        out=msg[:].rearrange("p (s h d) -> p s h d", h=H, d=D),
                        in0=sv[:, :, 0:HD].rearrange(
                            "p s (h d) -> p s h d", h=H),
                        in1=exv[:, :, :, None].to_broadcast([128, Kp, H, D]),
                        op=AG.mult)
                    num = ps.tile([128, HD], F32, tag="nacc")
                    for k in range(Kp):
                        nc.tensor.matmul(
                            num[:], lhsT=ident[:],
                            rhs=msg[:, k * HD:(k + 1) * HD],
                            start=(k == 0), stop=(k == Kp - 1))
                    if L < 2:
                        outw = sm.tile([128, HD], F32, tag="outw")
                        nc.vector.tensor_tensor(
                            out=outw[:].rearrange("p (h d) -> p h d", h=H),
                            in0=num[:].rearrange("p (h d) -> p h d", h=H),
                            in1=rec[:, :, None].to_broadcast([128, H, D]),
                            op=AG.mult)
                        r1 = sm.tile([128, HD], F32, tag="r1")
                        nc.scalar.activation(r1[:], outw[:],
                                             mybir.ActivationFunctionType.Relu)
                        m1 = sm.tile([128, HD], F32, tag="m1")
                        nc.scalar.activation(m1[:], outw[:],
                                             mybir.ActivationFunctionType.Relu,
                                             scale=-1.0)
                        em = sm.tile([128, HD], F32, tag="em")
                        nc.scalar.activation(em[:], m1[:],
                                             mybir.ActivationFunctionType.Exp,
                                             scale=-1.0)
                        xl = sm.tile([128, HD], BF16, tag="xl")
                        nc.vector.scalar_tensor_tensor(
                            out=xl[:], in0=em[:], scalar=-1.0, in1=r1[:],
                            op0=AG.add, op1=AG.add)
                        # PE-transpose the row block so the next dense phase
                        # reads plain (non-transposing) DMAs
                        xlt_ps = psT.tile([128, 128], F32, tag="tacc")
                        nc.tensor.matmul(xlt_ps[:], lhsT=xl[:], rhs=ident[:],
                                         start=True, stop=True)
                        xlt = sm.tile([128, 128], BF16, tag="xlt")
                        nc.scalar.copy(xlt[:], xlt_ps[:])
                        fw = nc.sync.dma_start(
                            xloc[L + 1][:, w * 128:(w + 1) * 128], xlt[:])
                    else:
                        prod = sm.tile([128, HD], F32, tag="prod")
                        nc.vector.tensor_tensor(
                            out=prod[:].rearrange("p (h c) -> p h c", h=H),
                            in0=num[:].rearrange("p (h c) -> p h c", h=H),
                            in1=rec[:, :, None].to_broadcast(
                                [128, H, NUM_CLASSES]),
                            op=AG.mult)
                        lg = sm.tile([128, NUM_CLASSES], F32, tag="lg")
                        nc.vector.tensor_reduce(
                            lg[:], prod[:].rearrange(
                                "p (h c) -> p c h", h=H),
                            axis=mybir.AxisListType.X, op=AG.add)
                        nc.vector.tensor_scalar_mul(lg[:], lg[:], 1.0 / H)
                        fw = nc.sync.dma_start(
                            out_t[w * 128:(w + 1) * 128, :], lg[:])
                    flush_writes.append(fw)
                nop_e = nc.gpsimd.engine_nop()
                for fwi in flush_writes:
                    add_dep_helper(nop_e.ins, fwi.ins)
                if L < n_layers - 1:
                    cc = nc.gpsimd.collective_compute(
                        "AllGather", AG.bypass,
                        [[i for i in range(N_CORES)]],
                        ins=[xloc[L + 1][:, :]], outs=[xfull[L + 1][:, :]])
                    nop_c = nc.vector.engine_nop()
                    add_dep_helper(nop_c.ins, cc.ins)
                    prev_nop = nop_c
                else:
                    prev_nop = nop_e
    nc.finalize()
    return nc


def kernel(**inputs):
    h = np.asarray(inputs["h"], np.float32)
    src = np.asarray(inputs["src"]).astype(np.int64)
    dst = np.asarray(inputs["dst"]).astype(np.int64)

    (core_lists, rank_of, Ks, offs, S_tot, idx16, mpar, mex,
     streams) = _host_prep(src, dst)

    # x0: h rows in rank order, bf16, padded
    x0 = np.zeros((NTOT, 256), ml_dtypes.bfloat16)
    for k in range(N_CORES):
        nodes = core_lists[k]
        x0[k * NPC:k * NPC + len(nodes)] = h[nodes].astype(ml_dtypes.bfloat16)

    # host pre-gather of the layer-0 slot stream, transposed for PE lhsT
    x0sT_hosts = [np.ascontiguousarray(x0[streams[k]].T)
                  for k in range(N_CORES)]
    Ws = [
        _fold_w(inputs["W0"], inputs["al0"], inputs["ar0"], 128, 4, 32, 256),
        _fold_w(inputs["W1"], inputs["al1"], inputs["ar1"], 128, 4, 32, 128),
        _fold_w(inputs["W2"], inputs["al2"], inputs["ar2"], 160, 4, 40, 128),
    ]

    nc = _build_program(Ks, offs, S_tot, None)

    in_maps = []
    for k in range(N_CORES):
        m = {"x0sT": x0sT_hosts[k], "w0": Ws[0], "w1": Ws[1],
             "w2": Ws[2], "idx16": idx16[k], "mpar": mpar[k],
             "mex": mex[k].astype(ml_dtypes.bfloat16),
             "ident": np.eye(128, dtype=ml_dtypes.bfloat16)}
        in_maps.append(m)

    trace = bool(int(os.environ.get("GAT_TRACE", "0")))
    res = run_bass_kernel_spmd(nc, in_maps, core_ids=list(range(N_CORES)),
                               trace=trace)
    LAST_EXEC_NS[0] = res.exec_time_ns
    LAST_RES[0] = res

    out = np.zeros((N_NODES, NUM_CLASSES), np.float32)
    for k in range(N_CORES):
        nodes = core_lists[k]
        out[nodes] = res.results[k]["logits"][:len(nodes)]
    return out

